# revision 1
# baseline (speedup 1.0000x reference)
"""LSTM regression kernel for 8 Trainium2 NeuronCores.

Model (reference): B=2048, IN=2048, H=1024, T=15 steps, x constant across
steps. Data-parallel over batch: each of the 8 cores handles 256 batch rows.

Device strategy (per core, batch BL=256):
 - Everything kept "transposed": state hT/cT stored as [H, BL] with H on
   partitions (8 chunks of 128), so no per-step transposes are needed.
 - gatesT[4H, BL] = W_hh @ hT accumulated in PSUM over 8 K-chunks, plus one
   extra identity-weight matmul that adds the precomputed xgT tile (this
   replaces a per-tile DVE add of the input-gate contribution).
 - xgT[4H, BL] = W_ihAug @ xAugT computed once at start; biases b_ih+b_hh are
   folded in host-side by augmenting x with a ones-row and W_ih with a bias
   row (padded to a whole 128-row chunk).
 - Activations (sigmoid/tanh) on ScalarE directly from PSUM; cell update on
   VectorE per 128-row h-chunk so it pipelines with the matmuls.
 - Matmul inputs in fp16 (fp32 PSUM accumulate) - all operands here are
   small-range, so fp16's 10-bit mantissa beats bf16 at identical PE speed.
   h kept in fp32 for output and re-cast to fp16 each step.
"""

import os
import numpy as np
import ml_dtypes

try:
    import concourse.bass as bass
except ImportError:  # pragma: no cover
    import sys
    sys.path.insert(0, "/opt/trn_rl_repo")
    import concourse.bass as bass
from concourse import bacc
import concourse.mybir as mybir
import concourse.tile as tile
from concourse.bass_utils import run_bass_kernel_spmd
from concourse.masks import make_identity

F32 = mybir.dt.float32
F16 = mybir.dt.float16
AF = mybir.ActivationFunctionType

T = 15
B, IN, H = 2048, 2048, 1024
NCORES = 8
BL = B // NCORES            # 256 batch rows per core
G4 = 4 * H                  # 4096 gate rows
NM = G4 // 128              # 32 gate m-tiles
NKH = H // 128              # 8 hidden K-chunks
INA = IN + 128              # x augmented with ones row, padded to chunk
NKX = INA // 128            # 17 input K-chunks
INIT = 0.01

LAST_EXEC_NS = None
LAST_RESULTS = None

_cached_nc = None


def _build():
    nc = bacc.Bacc(None, target_bir_lowering=False)
    wih_hi = nc.dram_tensor("wih_hi", [INA, G4], F16, kind="ExternalInput")
    wih_lo = nc.dram_tensor("wih_lo", [INA, G4], F16, kind="ExternalInput")
    whh = nc.dram_tensor("whh", [H, G4], F16, kind="ExternalInput")
    xt_hi = nc.dram_tensor("xt_hi", [INA, BL], F16, kind="ExternalInput")
    xt_lo = nc.dram_tensor("xt_lo", [INA, BL], F16, kind="ExternalInput")
    hs = nc.dram_tensor("hs", [T, 128, NKH * BL], F32, kind="ExternalOutput")

    with tile.TileContext(nc) as tc:
        with (
            tc.tile_pool(name="const", bufs=1) as constp,
            tc.tile_pool(name="wihp", bufs=4) as wihp,
            tc.tile_pool(name="state", bufs=2) as statep,
            tc.tile_pool(name="gates", bufs=3) as gatesp,
            tc.tile_pool(name="psum", bufs=8, space="PSUM") as psump,
        ):
            whh_sb = constp.tile([128, NKH * G4], F16, tag="whh")
            xg_hi = constp.tile([128, NM * BL], F16, tag="xghi")
            xg_lo = constp.tile([128, NM * BL], F16, tag="xglo")
            xth_sb = constp.tile([128, NKX * BL], F16, tag="xth")
            xtl_sb = constp.tile([128, NKX * BL], F16, tag="xtl")
            ident = constp.tile([128, 128], F16, tag="ident")
            make_identity(nc, ident[:, :])

            whh_r = whh[:, :].rearrange("(kc p) m -> kc p m", p=128)
            for kc in range(NKH):
                nc.sync.dma_start(whh_sb[:, kc * G4:(kc + 1) * G4], whh_r[kc])
            xth_r = xt_hi[:, :].rearrange("(kc p) b -> kc p b", p=128)
            xtl_r = xt_lo[:, :].rearrange("(kc p) b -> kc p b", p=128)
            for kc in range(NKX):
                nc.sync.dma_start(xth_sb[:, kc * BL:(kc + 1) * BL], xth_r[kc])
                nc.sync.dma_start(xtl_sb[:, kc * BL:(kc + 1) * BL], xtl_r[kc])

            # ---- xg phase: 4 sweeps, each producing ALL 4 gates for an
            # hc-pair (so recurrent step 0 for hc 0..1 can start after the
            # first sweep and overlap the rest of the xg phase) ----
            for sweep in range(4):
                pstiles = [psump.tile([128, BL], F32, tag="ps", name=f"psxg{i}") for i in range(8)]
                for kc in range(NKX):
                    wth = wihp.tile([128, 1024], F16, tag="wihh", name="wth")
                    wtl = wihp.tile([128, 1024], F16, tag="wihl", name="wtl")
                    src_h = wih_hi[kc * 128:(kc + 1) * 128, :].rearrange(
                        "p (g t c) -> p g t c", g=4, t=4
                    )[:, :, sweep, :]
                    src_l = wih_lo[kc * 128:(kc + 1) * 128, :].rearrange(
                        "p (g t c) -> p g t c", g=4, t=4
                    )[:, :, sweep, :]
                    nc.sync.dma_start(wth[:, :], src_h)
                    nc.sync.dma_start(wtl[:, :], src_l)
                    for ml in range(8):
                        for pi, (wt_, xt_) in enumerate(
                            [(wth, xth_sb), (wth, xtl_sb), (wtl, xth_sb)]
                        ):
                            nc.tensor.matmul(
                                pstiles[ml][:, :],
                                wt_[:, ml * 128:(ml + 1) * 128],
                                xt_[:, kc * BL:(kc + 1) * BL],
                                start=(kc == 0 and pi == 0),
                                stop=(kc == NKX - 1 and pi == 2),
                            )
                for ml in range(8):
                    g_, j_ = ml // 2, ml % 2
                    m = g_ * 8 + sweep * 2 + j_
                    nc.scalar.copy(xg_hi[:, m * BL:(m + 1) * BL], pstiles[ml][:, :])
                    nc.vector.tensor_sub(
                        xg_lo[:, m * BL:(m + 1) * BL],
                        pstiles[ml][:, :],
                        xg_hi[:, m * BL:(m + 1) * BL],
                    )

            # ---- recurrent steps ----
            h_prev = statep.tile([128, NKH * BL], F16, tag="hbf")
            c_prev = statep.tile([128, NKH * BL], F32, tag="c")
            nc.any.memset(h_prev[:, :], INIT)
            nc.any.memset(c_prev[:, :], INIT)

            for t in range(T):
                h_bf = statep.tile([128, NKH * BL], F16, tag="hbf")
                h_f32 = statep.tile([128, NKH * BL], F32, tag="hf")
                c_new = statep.tile([128, NKH * BL], F32, tag="c")
                for hc in range(NKH):
                    gt = []
                    for gi in range(4):
                        m = gi * NKH + hc
                        ps = psump.tile([128, BL], F32, tag="ps", name="psrec")
                        for kc in range(NKH):
                            nc.tensor.matmul(
                                ps[:, :],
                                whh_sb[:, kc * G4 + m * 128: kc * G4 + (m + 1) * 128],
                                h_prev[:, kc * BL:(kc + 1) * BL],
                                start=(kc == 0),
                                stop=False,
                            )
                        nc.tensor.matmul(
                            ps[:, :],
                            ident[:, :],
                            xg_hi[:, m * BL:(m + 1) * BL],
                            start=False,
                            stop=False,
                        )
                        nc.tensor.matmul(
                            ps[:, :],
                            ident[:, :],
                            xg_lo[:, m * BL:(m + 1) * BL],
                            start=False,
                            stop=True,
                        )
                        g = gatesp.tile([128, BL], F32, tag=f"g{gi}", name=f"gate{gi}")
                        fn = AF.Tanh if gi == 2 else AF.Sigmoid
                        nc.scalar.activation(g[:, :], ps[:, :], fn)
                        gt.append(g)
                    sl = slice(hc * BL, (hc + 1) * BL)
                    t0 = gatesp.tile([128, BL], F32, tag="t0")
                    t1 = gatesp.tile([128, BL], F32, tag="t1")
                    th = gatesp.tile([128, BL], F32, tag="th")
                    nc.vector.tensor_mul(t0[:, :], gt[0][:, :], gt[2][:, :])
                    nc.vector.tensor_mul(t1[:, :], gt[1][:, :], c_prev[:, sl])
                    nc.vector.tensor_add(c_new[:, sl], t0[:, :], t1[:, :])
                    nc.scalar.activation(th[:, :], c_new[:, sl], AF.Tanh)
                    nc.vector.tensor_mul(h_f32[:, sl], gt[3][:, :], th[:, :])
                    nc.vector.tensor_copy(h_bf[:, sl], h_f32[:, sl])
                nc.sync.dma_start(hs[t], h_f32[:, :])
                h_prev, c_prev = h_bf, c_new

    nc.compile()
    return nc


def timeline_ns():
    from concourse.timeline_sim import TimelineSim
    nc = _get_nc()
    ts = TimelineSim(nc)
    ts.simulate()
    return ts.time


def _get_nc():
    global _cached_nc
    if _cached_nc is None:
        _cached_nc = _build()
    return _cached_nc


def kernel(x, W_ih, W_hh, b_ih, b_hh):
    global LAST_EXEC_NS, LAST_RESULTS
    nc = _get_nc()
    bf = np.float16
    x = np.asarray(x, np.float32)
    W_ih = np.asarray(W_ih, np.float32)
    W_hh = np.asarray(W_hh, np.float32)
    b_ih = np.asarray(b_ih, np.float32)
    b_hh = np.asarray(b_hh, np.float32)

    def hilo(a):
        hi = a.astype(bf)
        lo = (a - hi.astype(np.float32)).astype(bf)
        return hi, lo

    waug = np.zeros((INA, G4), np.float32)
    waug[:IN] = W_ih.T
    waug[IN] = b_ih + b_hh
    waug_hi, waug_lo = hilo(waug)
    whh_bf = np.ascontiguousarray(W_hh.T).astype(bf)

    in_maps = []
    for c in range(NCORES):
        xa = np.zeros((INA, BL), np.float32)
        xa[:IN] = x[c * BL:(c + 1) * BL].T
        xa[IN] = 1.0
        xa_hi, xa_lo = hilo(xa)
        in_maps.append({
            "wih_hi": waug_hi, "wih_lo": waug_lo, "whh": whh_bf,
            "xt_hi": xa_hi, "xt_lo": xa_lo,
        })

    trace = os.environ.get("LSTM_TRACE") == "1"
    res = run_bass_kernel_spmd(
        nc, in_maps, core_ids=list(range(NCORES)), trace=trace
    )
    LAST_EXEC_NS = res.exec_time_ns
    LAST_RESULTS = res

    out = np.empty((T, B, H), np.float32)
    for c in range(NCORES):
        a = res.results[c]["hs"].reshape(T, 128, NKH, BL)
        out[:, c * BL:(c + 1) * BL, :] = (
            a.transpose(0, 3, 2, 1).reshape(T, BL, H)
        )
    return out



# revision 4
# speedup vs baseline: 1.5397x; 1.5397x over previous
"""LSTM regression kernel for 8 Trainium2 NeuronCores.

Model (reference): B=2048, IN=2048, H=1024, T=15 steps, x constant across
steps. Data-parallel over batch: each of the 8 cores handles 256 batch rows.

Device strategy (per core, batch BL=256):
 - Everything kept "transposed": state hT/cT stored as [H, BL] with H on
   partitions (8 chunks of 128), so no per-step transposes are needed.
 - Phase A (fused xg + step 0): xgT[4H, BL] = W_ihAug @ xAugT in a single
   fp16 pass (biases b_ih+b_hh folded host-side via an augmented ones-row).
   Since h0 is the constant 0.01, step 0's recurrent term is the per-row
   constant r = 0.01*rowsum(W_hh), folded into the gate activation as the
   per-partition bias operand -- step 0 needs NO recurrent matmuls.
 - Phase B (steps 1..14): gatesT[4H, BL] = W_hh @ hT accumulated in PSUM
   over 8 K-chunks; the precomputed xg tile is added in-place into PSUM on
   the DVE (cheaper than identity-weight matmuls on the busy PE).
 - Activations (sigmoid/tanh) on ScalarE directly from PSUM; cell update on
   VectorE per 128-row h-chunk so it pipelines with the matmuls.
 - Matmul inputs in fp16 (fp32 PSUM accumulate); h kept fp16; hidden-state
   outputs stored fp16 and upconverted host-side.
"""

import os
import numpy as np
import ml_dtypes

try:
    import concourse.bass as bass
except ImportError:  # pragma: no cover
    import sys
    sys.path.insert(0, "/opt/trn_rl_repo")
    import concourse.bass as bass
from concourse import bacc
import concourse.mybir as mybir
import concourse.tile as tile
from concourse.bass_utils import run_bass_kernel_spmd

F32 = mybir.dt.float32
F16 = mybir.dt.float16
AF = mybir.ActivationFunctionType
ALU = mybir.AluOpType

T = 15
B, IN, H = 2048, 2048, 1024
NCORES = 8
BL = B // NCORES            # 256 batch rows per core
G4 = 4 * H                  # 4096 gate rows
NM = G4 // 128              # 32 gate m-tiles
NKH = H // 128              # 8 hidden K-chunks
NKX = IN // 128             # 16 input K-chunks (biases folded via ACT bias)
NQ = 4                      # wih quad-chunks per sweep (4 kc each)
NSW = 4                     # sweeps; sweep s covers hc pair (2s, 2s+1)
INIT = 0.01

LAST_EXEC_NS = None
LAST_RESULTS = None

_cached_nc = None


def _build():
    nc = bacc.Bacc(None, target_bir_lowering=False)
    wih = nc.dram_tensor("wih", [NSW, NKX, 128, 1024], F16, kind="ExternalInput")
    whh = nc.dram_tensor("whh", [NKH, 128, G4], F16, kind="ExternalInput")
    xt = nc.dram_tensor("xt", [NKX, 128, BL], F16, kind="ExternalInput")
    rb = nc.dram_tensor("rb", [128, NM], F32, kind="ExternalInput")
    hs = nc.dram_tensor("hs", [T, 128, NKH * BL], F16, kind="ExternalOutput")

    with tile.TileContext(nc) as tc:
        with (
            tc.tile_pool(name="const", bufs=1) as constp,
            tc.tile_pool(name="wihp", bufs=6) as wihp,
            tc.tile_pool(name="state", bufs=2) as statep,
            tc.tile_pool(name="gates", bufs=3) as gatesp,
            tc.tile_pool(name="psum", bufs=8, space="PSUM") as psump,
        ):
            xt_sb = constp.tile([128, NKX * BL], F16, tag="xt")
            r_sb = constp.tile([128, NM], F32, tag="rb")
            xg_sb = constp.tile([128, NM * BL], F32, tag="xg")
            whh_sb = [
                constp.tile([128, G4], F16, tag=f"whh{kc}", name=f"whh{kc}")
                for kc in range(NKH)
            ]

            for kc in range(NKX):
                nc.sync.dma_start(xt_sb[:, kc * BL:(kc + 1) * BL], xt[kc])
            nc.sync.dma_start(r_sb[:, :], rb[:, :])

            # ---- Phase A: xg = W_aug @ x_aug (single fp16 pass) fused with
            # step 0 (bias trick for the constant-h0 recurrent term).
            h0 = statep.tile([128, NKH * BL], F16, tag="h")
            c0 = statep.tile([128, NKH * BL], F32, tag="c")
            for s in range(NSW):
                pst = [
                    psump.tile([128, BL], F32, tag="ps", name=f"psA{i}")
                    for i in range(8)
                ]
                for kc in range(NKX):
                    wt = wihp.tile([128, 1024], F16, tag="wih", name="wt")
                    nc.sync.dma_start(wt[:, :], wih[s, kc])
                    for ml in range(8):
                        nc.tensor.matmul(
                            pst[ml][:, :],
                            wt[:, ml * 128:(ml + 1) * 128],
                            xt_sb[:, kc * BL:(kc + 1) * BL],
                            start=(kc == 0),
                            stop=(kc == NKX - 1),
                        )
                g0 = {}
                for ml in range(8):
                    gi, j = ml // 2, ml % 2
                    m = gi * NKH + 2 * s + j
                    nc.vector.tensor_copy(
                        xg_sb[:, m * BL:(m + 1) * BL], pst[ml][:, :]
                    )
                    g = gatesp.tile([128, BL], F32, tag=f"g{gi}", name=f"gA{gi}{j}")
                    fn = AF.Tanh if gi == 2 else AF.Sigmoid
                    nc.scalar.activation(
                        g[:, :], pst[ml][:, :], fn, bias=r_sb[:, m:m + 1]
                    )
                    g0[(gi, j)] = g
                for j in range(2):
                    hc = 2 * s + j
                    sl = slice(hc * BL, (hc + 1) * BL)
                    t0 = gatesp.tile([128, BL], F32, tag="t0")
                    th = gatesp.tile([128, BL], F32, tag="th")
                    nc.vector.tensor_mul(t0[:, :], g0[(0, j)][:, :], g0[(2, j)][:, :])
                    # c0 = f*INIT + i*g  (c_prev is the 0.01 constant)
                    nc.vector.scalar_tensor_tensor(
                        c0[:, sl], g0[(1, j)][:, :], INIT, t0[:, :],
                        ALU.mult, ALU.add,
                    )
                    nc.scalar.activation(th[:, :], c0[:, sl], AF.Tanh)
                    nc.vector.tensor_mul(h0[:, sl], g0[(3, j)][:, :], th[:, :])
                # stream W_hh in behind the wih sweeps (2 chunks per sweep)
                for kc in (2 * s, 2 * s + 1):
                    nc.sync.dma_start(whh_sb[kc][:, :], whh[kc])
            nc.sync.dma_start(hs[0], h0[:, :])

            # ---- Phase B: recurrent steps 1..14 ----
            h_prev, c_prev = h0, c0
            for t in range(1, T):
                h_new = statep.tile([128, NKH * BL], F16, tag="h")
                c_new = statep.tile([128, NKH * BL], F32, tag="c")
                for hc in range(NKH):
                    gt = []
                    for gi in range(4):
                        m = gi * NKH + hc
                        ps = psump.tile([128, BL], F32, tag="ps", name="psB")
                        for kc in range(NKH):
                            nc.tensor.matmul(
                                ps[:, :],
                                whh_sb[kc][:, m * 128:(m + 1) * 128],
                                h_prev[:, kc * BL:(kc + 1) * BL],
                                start=(kc == 0),
                                stop=(kc == NKH - 1),
                            )
                        nc.vector.tensor_add(
                            ps[:, :], ps[:, :], xg_sb[:, m * BL:(m + 1) * BL]
                        )
                        g = gatesp.tile([128, BL], F32, tag=f"g{gi}", name=f"gB{gi}")
                        fn = AF.Tanh if gi == 2 else AF.Sigmoid
                        nc.scalar.activation(g[:, :], ps[:, :], fn)
                        gt.append(g)
                    sl = slice(hc * BL, (hc + 1) * BL)
                    t0 = gatesp.tile([128, BL], F32, tag="t0")
                    t1 = gatesp.tile([128, BL], F32, tag="t1")
                    th = gatesp.tile([128, BL], F32, tag="th")
                    nc.vector.tensor_mul(t0[:, :], gt[0][:, :], gt[2][:, :])
                    nc.vector.tensor_mul(t1[:, :], gt[1][:, :], c_prev[:, sl])
                    nc.vector.tensor_add(c_new[:, sl], t0[:, :], t1[:, :])
                    nc.scalar.activation(th[:, :], c_new[:, sl], AF.Tanh)
                    nc.vector.tensor_mul(h_new[:, sl], gt[3][:, :], th[:, :])
                nc.sync.dma_start(hs[t], h_new[:, :])
                h_prev, c_prev = h_new, c_new

    nc.compile()
    return nc


def timeline_ns():
    from concourse.timeline_sim import TimelineSim
    nc = _get_nc()
    ts = TimelineSim(nc)
    ts.simulate()
    return ts.time


def _get_nc():
    global _cached_nc
    if _cached_nc is None:
        _cached_nc = _build()
    return _cached_nc


def _pack_weights(W_ih, W_hh, b_ih, b_hh):
    f16 = np.float16
    waug = np.zeros((INA, G4), np.float32)
    waug[:IN] = W_ih.T
    waug[IN] = b_ih + b_hh
    # [kc, p, gi, s, j, col] -> [s, kc, p, gi, j, col]
    wperm = waug.reshape(NKX, 128, 4, NSW, 2, 128).transpose(3, 0, 1, 2, 4, 5)
    wih_host = np.ascontiguousarray(wperm.reshape(NSW, NKX, 128, 1024)).astype(f16)
    whh_host = np.ascontiguousarray(W_hh.T.reshape(NKH, 128, G4)).astype(f16)
    r = (INIT * W_hh.sum(axis=1)).astype(np.float32)        # [4096]
    r_host = np.ascontiguousarray(r.reshape(NM, 128).T)     # [128, 32]
    return wih_host, whh_host, r_host


def kernel(x, W_ih, W_hh, b_ih, b_hh):
    global LAST_EXEC_NS, LAST_RESULTS
    nc = _get_nc()
    x = np.asarray(x, np.float32)
    W_ih = np.asarray(W_ih, np.float32)
    W_hh = np.asarray(W_hh, np.float32)
    b_ih = np.asarray(b_ih, np.float32)
    b_hh = np.asarray(b_hh, np.float32)

    wih_host, whh_host, r_host = _pack_weights(W_ih, W_hh, b_ih, b_hh)

    in_maps = []
    for c in range(NCORES):
        xa = np.zeros((INA, BL), np.float32)
        xa[:IN] = x[c * BL:(c + 1) * BL].T
        xa[IN] = 1.0
        xt_host = xa.astype(np.float16).reshape(NKX, 128, BL)
        in_maps.append({
            "wih": wih_host, "whh": whh_host, "xt": xt_host, "rb": r_host,
        })

    trace = os.environ.get("LSTM_TRACE") == "1"
    res = run_bass_kernel_spmd(
        nc, in_maps, core_ids=list(range(NCORES)), trace=trace
    )
    LAST_EXEC_NS = res.exec_time_ns
    LAST_RESULTS = res

    out = np.empty((T, B, H), np.float32)
    for c in range(NCORES):
        a = res.results[c]["hs"].astype(np.float32).reshape(T, 128, NKH, BL)
        out[:, c * BL:(c + 1) * BL, :] = (
            a.transpose(0, 3, 2, 1).reshape(T, BL, H)
        )
    return out


# revision 25
# speedup vs baseline: 1.6097x; 1.0454x over previous
"""LSTM regression kernel for 8 Trainium2 NeuronCores.

Model (reference): B=2048, IN=2048, H=1024, T=15 steps, x constant across
steps. Data-parallel over batch: each of the 8 cores handles 256 batch rows.

Device strategy (per core, batch BL=256):
 - Everything kept "transposed": state hT/cT stored as [H, BL] with H on
   partitions (8 chunks of 128), so no per-step transposes are needed.
 - Phase A (fused xg + step 0): xgT[4H, BL] = W_ihAug @ xAugT in a single
   fp16 pass (biases b_ih+b_hh folded host-side via an augmented ones-row).
   Since h0 is the constant 0.01, step 0's recurrent term is the per-row
   constant r = 0.01*rowsum(W_hh), folded into the gate activation as the
   per-partition bias operand -- step 0 needs NO recurrent matmuls.
 - Phase B (steps 1..14): gatesT[4H, BL] = W_hh @ hT accumulated in PSUM
   over 8 K-chunks; the precomputed xg tile is added in-place into PSUM on
   the DVE (cheaper than identity-weight matmuls on the busy PE).
 - Activations (sigmoid/tanh) on ScalarE directly from PSUM; cell update on
   VectorE per 128-row h-chunk so it pipelines with the matmuls.
 - Matmul inputs in fp16 (fp32 PSUM accumulate); h kept fp16; hidden-state
   outputs stored fp16 and upconverted host-side.
"""

import os
import numpy as np
import ml_dtypes

try:
    import concourse.bass as bass
except ImportError:  # pragma: no cover
    import sys
    sys.path.insert(0, "/opt/trn_rl_repo")
    import concourse.bass as bass
from concourse import bacc
import concourse.mybir as mybir
import concourse.tile as tile
from concourse.bass_utils import run_bass_kernel_spmd

F32 = mybir.dt.float32
F16 = mybir.dt.float16
AF = mybir.ActivationFunctionType
ALU = mybir.AluOpType

T = 15
B, IN, H = 2048, 2048, 1024
NCORES = 8
BL = B // NCORES            # 256 batch rows per core
G4 = 4 * H                  # 4096 gate rows
NM = G4 // 128              # 32 gate m-tiles
NKH = H // 128              # 8 hidden K-chunks
NKX = IN // 128             # 16 input K-chunks (biases folded via ACT bias)
NQ = 4                      # wih quad-chunks per sweep (4 kc each)
NSW = 4                     # sweeps; sweep s covers hc pair (2s, 2s+1)
INIT = 0.01

LAST_EXEC_NS = None
LAST_RESULTS = None

_cached_nc = None


def _build():
    nc = bacc.Bacc(None, target_bir_lowering=False)
    wih = nc.dram_tensor("wih", [NSW, NQ, 128, 4 * 1024], F16, kind="ExternalInput")
    whh = nc.dram_tensor("whh", [NKH, 128, G4], F16, kind="ExternalInput")
    xt = nc.dram_tensor("xt", [NKX, 128, BL], F16, kind="ExternalInput")
    rb = nc.dram_tensor("rb", [128, NM], F32, kind="ExternalInput")
    bb = nc.dram_tensor("bb", [128, NM], F32, kind="ExternalInput")
    hs = nc.dram_tensor("hs", [T, 128, NKH * BL], F16, kind="ExternalOutput")

    with tile.TileContext(nc) as tc:
        with (
            tc.tile_pool(name="const", bufs=1) as constp,
            tc.tile_pool(name="wihp", bufs=4) as wihp,
            tc.tile_pool(name="w0p", bufs=2) as w0p,
            tc.tile_pool(name="state", bufs=2) as statep,
            tc.tile_pool(name="gates", bufs=3) as gatesp,
            tc.tile_pool(name="psum", bufs=8, space="PSUM") as psump,
        ):
            xt_sbq = [
                constp.tile([128, 4 * BL], F16, tag=f"xtq{q}", name=f"xtq{q}")
                for q in range(4)
            ]
            r_sb = constp.tile([128, NM], F32, tag="rb")
            bb_sb = constp.tile([128, NM], F32, tag="bb")
            xg_sb = constp.tile([128, NM * BL], F32, tag="xg")
            whh_sb = [
                constp.tile([128, G4], F16, tag=f"whh{kc}", name=f"whh{kc}")
                for kc in range(NKH)
            ]

            # Sweep 0's wih as 16 single-kc tiles interleaved with the x
            # quarters: the DMA unit transfers serially (~0.7us per 256KB),
            # so fine-grained tiles keep delivery just ahead of the PE's
            # ~0.93us/chunk consumption from the very first matmul.
            w0s = [
                w0p.tile([128, 1024], F16, tag=f"w0s{i % 4}", name=f"w0s_{i}")
                for i in range(NKX)
            ]

            def _dma_w0(i):
                nc.sync.dma_start(
                    w0s[i][:, :], wih[0, i // 4][:, (i % 4) * 1024:
                                                 (i % 4 + 1) * 1024]
                )

            def _dma_xtq(q):
                nc.sync.dma_start(
                    xt_sbq[q][:, :].rearrange("p (kc b) -> p kc b", kc=4),
                    xt[4 * q:4 * q + 4].rearrange("kc p b -> p kc b"),
                )

            _dma_w0(0)
            _dma_xtq(0)
            for i in (1, 2, 3):
                _dma_w0(i)
            _dma_xtq(1)
            for i in (4, 5, 6, 7):
                _dma_w0(i)
            _dma_xtq(2)
            for i in (8, 9, 10, 11):
                _dma_w0(i)
            _dma_xtq(3)
            for i in (12, 13, 14, 15):
                _dma_w0(i)
            # first sweep-1 quad jumps ahead of r/bb (needed ~3us earlier)
            wq1 = wihp.tile([128, 4 * 1024], F16, tag="wih", name="wq1")
            nc.sync.dma_start(wq1[:, :], wih[1, 0])
            nc.sync.dma_start(r_sb[:, :], rb[:, :])
            nc.sync.dma_start(bb_sb[:, :], bb[:, :])

            # ---- Phase A: xg = W_ih @ x (single fp16 pass) fused with
            # step 0 (per-partition ACT bias carries b_ih+b_hh and the
            # constant-h0 recurrent term 0.01*rowsum(W_hh)).
            # PSUM tiles pack the sweep's hc-pair per gate ([128, 512] = one
            # bank) so two sweeps can double-buffer across the 8 banks.
            h0 = statep.tile([128, NKH * BL], F16, tag="h")
            c0 = statep.tile([128, NKH * BL], F32, tag="c")
            for s in range(NSW):
                # one [128,256] PSUM tile per (gate, hc-of-pair): a PSUM bank
                # admits only ONE pending accumulation group (2KB zero
                # region), so tiles must not share banks across groups.
                pst = [
                    psump.tile([128, BL], F32, tag="ps", name=f"psA{i}")
                    for i in range(8)
                ]
                for qc in range(NQ):
                    first = s == 0
                    if not first:
                        if s == 1 and qc == 0:
                            wq = wq1
                        else:
                            wq = wihp.tile(
                                [128, 4 * 1024], F16, tag="wih", name="wq"
                            )
                            nc.sync.dma_start(wq[:, :], wih[s, qc])
                    for kci in range(4):
                        kc = qc * 4 + kci
                        for ml in range(8):
                            lhs = (
                                w0s[kc][:, ml * 128:(ml + 1) * 128]
                                if first else
                                wq[:, kci * 1024 + ml * 128:
                                   kci * 1024 + (ml + 1) * 128]
                            )
                            nc.tensor.matmul(
                                pst[ml][:, :],
                                lhs,
                                xt_sbq[qc][:, kci * BL:(kci + 1) * BL],
                                start=(kc == 0),
                                stop=(kc == NKX - 1),
                            )
                g0 = {}
                for ml in range(8):
                    gi, j = ml // 2, ml % 2
                    q = (2 * s + j) * 4 + gi
                    # xg tile saved with biases folded in; on DVE (idle in
                    # phase A) so the PSUM slot frees without waiting on the
                    # ACT queue at sweep boundaries
                    nc.vector.tensor_scalar_add(
                        xg_sb[:, q * BL:(q + 1) * BL],
                        pst[ml][:, :],
                        bb_sb[:, q:q + 1],
                    )
                    g = gatesp.tile([128, BL], F32, tag=f"g{gi}", name=f"gA{gi}{j}")
                    fn = AF.Tanh if gi == 2 else AF.Sigmoid
                    nc.scalar.activation(
                        g[:, :], pst[ml][:, :], fn,
                        bias=r_sb[:, q:q + 1],
                    )
                    g0[(gi, j)] = g
                for j in range(2):
                    hc = 2 * s + j
                    sl = slice(hc * BL, (hc + 1) * BL)
                    t0 = gatesp.tile([128, BL], F32, tag="t0")
                    th = gatesp.tile([128, BL], F32, tag="th")
                    nc.vector.tensor_mul(t0[:, :], g0[(0, j)][:, :], g0[(2, j)][:, :])
                    # c0 = f*INIT + i*g  (c_prev is the 0.01 constant)
                    nc.vector.scalar_tensor_tensor(
                        c0[:, sl], g0[(1, j)][:, :], INIT, t0[:, :],
                        ALU.mult, ALU.add,
                    )
                    nc.scalar.activation(th[:, :], c0[:, sl], AF.Tanh)
                    nc.vector.tensor_mul(h0[:, sl], g0[(3, j)][:, :], th[:, :])
            # W_hh streams in right after the wih tiles, earliest chunk first
            for kc in range(NKH):
                nc.sync.dma_start(whh_sb[kc][:, :], whh[kc])
            nc.sync.dma_start(hs[0], h0[:, :])

            # ---- Phase B: recurrent steps 1..14 ----
            def _cell(gt, hc, c_prev, c_new, h_new):
                sl = slice(hc * BL, (hc + 1) * BL)
                t0 = gatesp.tile([128, BL], F32, tag="t0", name="t0")
                t1 = gatesp.tile([128, BL], F32, tag="t1", name="t1")
                th = gatesp.tile([128, BL], F32, tag="th", name="th")
                nc.vector.tensor_mul(t0[:, :], gt[0][:, :], gt[2][:, :])
                nc.vector.tensor_mul(t1[:, :], gt[1][:, :], c_prev[:, sl])
                nc.vector.tensor_add(c_new[:, sl], t0[:, :], t1[:, :])
                nc.scalar.activation(th[:, :], c_new[:, sl], AF.Tanh)
                nc.vector.tensor_mul(h_new[:, sl], gt[3][:, :], th[:, :])

            h_prev, c_prev = h0, c0

            for t in range(1, T):
                h_new = statep.tile([128, NKH * BL], F16, tag="h")
                c_new = statep.tile([128, NKH * BL], F32, tag="c")
                for hc in range(NKH):
                    pss = []
                    for gi in range(4):
                        m = gi * NKH + hc
                        ps = psump.tile([128, BL], F32, tag="ps", name="psB")
                        # For the first gate pair of a step, defer the kc=7
                        # chunk: the previous step's h[7] lands ~1us after
                        # its last matmul, so give the PE runway.
                        kcs = (
                            list(range(NKH - 1)) if hc == 0 and gi < 2
                            else list(range(NKH))
                        )
                        for kc in kcs:
                            nc.tensor.matmul(
                                ps[:, :],
                                whh_sb[kc][:, m * 128:(m + 1) * 128],
                                h_prev[:, kc * BL:(kc + 1) * BL],
                                start=(kc == 0),
                                stop=(kc == NKH - 1),
                            )
                        pss.append(ps)
                    if hc == 0:
                        for gi in range(2):
                            m = gi * NKH
                            nc.tensor.matmul(
                                pss[gi][:, :],
                                whh_sb[NKH - 1][:, m * 128:(m + 1) * 128],
                                h_prev[:, (NKH - 1) * BL:NKH * BL],
                                start=False,
                                stop=True,
                            )
                    gt = []
                    for gi in range(4):
                        q = hc * 4 + gi
                        ps = pss[gi]
                        nc.vector.tensor_add(
                            ps[:, :], ps[:, :], xg_sb[:, q * BL:(q + 1) * BL]
                        )
                        g = gatesp.tile([128, BL], F32, tag=f"g{gi}", name=f"gB{gi}")
                        fn = AF.Tanh if gi == 2 else AF.Sigmoid
                        nc.scalar.activation(g[:, :], ps[:, :], fn)
                        gt.append(g)
                    _cell(gt, hc, c_prev, c_new, h_new)
                nc.sync.dma_start(hs[t], h_new[:, :])
                h_prev, c_prev = h_new, c_new

    nc.compile()
    return nc


def timeline_ns():
    from concourse.timeline_sim import TimelineSim
    nc = _get_nc()
    ts = TimelineSim(nc)
    ts.simulate()
    return ts.time


def _get_nc():
    global _cached_nc
    if _cached_nc is None:
        _cached_nc = _build()
    return _cached_nc


def _pack_weights(W_ih, W_hh, b_ih, b_hh):
    f16 = np.float16
    wt = W_ih.T.astype(np.float32)                          # [IN, 4H]
    # [kc, p, gi, s, j, col] -> [s, kc, p, gi, j, col]
    wperm = wt.reshape(NKX, 128, 4, NSW, 2, 128).transpose(3, 0, 1, 2, 4, 5)
    # regroup kc into quads: [s, qc, kci, p, gi, j, col] -> [s, qc, p, kci, ...]
    wperm = wperm.reshape(NSW, NQ, 4, 128, 1024).transpose(0, 1, 3, 2, 4)
    wih_host = np.ascontiguousarray(
        wperm.reshape(NSW, NQ, 128, 4 * 1024)
    ).astype(f16)
    whh_host = np.ascontiguousarray(W_hh.T.reshape(NKH, 128, G4)).astype(f16)
    bias = (b_ih + b_hh).astype(np.float32)                 # [4096]
    r = (INIT * W_hh.sum(axis=1)).astype(np.float32) + bias

    def _qlay(v):  # [4096] in (gi, hc, p) order -> [128, q=hc*4+gi]
        return np.ascontiguousarray(
            v.reshape(4, NKH, 128).transpose(1, 0, 2).reshape(NM, 128).T
        )

    return wih_host, whh_host, _qlay(r), _qlay(bias)


def kernel(x, W_ih, W_hh, b_ih, b_hh):
    global LAST_EXEC_NS, LAST_RESULTS
    nc = _get_nc()
    x = np.asarray(x, np.float32)
    W_ih = np.asarray(W_ih, np.float32)
    W_hh = np.asarray(W_hh, np.float32)
    b_ih = np.asarray(b_ih, np.float32)
    b_hh = np.asarray(b_hh, np.float32)

    wih_host, whh_host, r_host, b_host = _pack_weights(W_ih, W_hh, b_ih, b_hh)

    in_maps = []
    for c in range(NCORES):
        xa = np.ascontiguousarray(x[c * BL:(c + 1) * BL].T)
        xt_host = xa.astype(np.float16).reshape(NKX, 128, BL)
        in_maps.append({
            "wih": wih_host, "whh": whh_host, "xt": xt_host,
            "rb": r_host, "bb": b_host,
        })

    trace = os.environ.get("LSTM_TRACE") == "1"
    res = run_bass_kernel_spmd(
        nc, in_maps, core_ids=list(range(NCORES)), trace=trace
    )
    LAST_EXEC_NS = res.exec_time_ns
    LAST_RESULTS = res

    out = np.empty((T, B, H), np.float32)
    for c in range(NCORES):
        a = res.results[c]["hs"].astype(np.float32).reshape(T, 128, NKH, BL)
        out[:, c * BL:(c + 1) * BL, :] = (
            a.transpose(0, 3, 2, 1).reshape(T, BL, H)
        )
    return out


# revision 30
# speedup vs baseline: 1.6188x; 1.0056x over previous
"""LSTM regression kernel for 8 Trainium2 NeuronCores.

Model (reference): B=2048, IN=2048, H=1024, T=15 steps, x constant across
steps. Data-parallel over batch: each of the 8 cores handles 256 batch rows.

Device strategy (per core, batch BL=256):
 - Everything kept "transposed": state hT/cT stored as [H, BL] with H on
   partitions (8 chunks of 128), so no per-step transposes are needed.
 - Phase A (fused xg + step 0): xgT[4H, BL] = W_ih @ xT in a single fp16
   pass. The gate bias b_ih+b_hh rides along as the per-partition scalar
   operand of the DVE op that saves xg to SBUF. Since h0 is the constant
   0.01, step 0's recurrent term is the per-row constant 0.01*rowsum(W_hh),
   folded into the step-0 gate activations as the per-partition ACT bias --
   step 0 needs NO recurrent matmuls.
 - Phase B (steps 1..14): gatesT[4H, BL] = W_hh @ hT accumulated in PSUM
   over 8 K-chunks; the precomputed xg tile is added in-place into PSUM on
   the DVE (cheaper than identity-weight matmuls on the busy PE).
 - Activations (sigmoid/tanh) on ScalarE directly from PSUM; cell update on
   VectorE per 128-row h-chunk so it pipelines with the matmuls.
 - Matmul inputs in fp16 (fp32 PSUM accumulate); h kept fp16; hidden-state
   outputs stored fp16 and upconverted host-side.
 - DMA choreography matters as much as the engines: the DMA unit moves
   ~0.36 GB/ms serially, so weight tiles are sized/ordered so delivery just
   leads consumption (sweep-0 singles first, W_hh streamed behind the wih
   sweeps, consumed via a kc7-deferral right after the phase switch).
"""

import os
import numpy as np

try:
    import concourse.bass as bass
except ImportError:  # pragma: no cover
    import sys
    sys.path.insert(0, "/opt/trn_rl_repo")
    import concourse.bass as bass
from concourse import bacc
import concourse.mybir as mybir
import concourse.tile as tile
from concourse.bass_utils import run_bass_kernel_spmd

F32 = mybir.dt.float32
F16 = mybir.dt.float16
AF = mybir.ActivationFunctionType
ALU = mybir.AluOpType

T = 15
B, IN, H = 2048, 2048, 1024
NCORES = 8
BL = B // NCORES            # 256 batch rows per core
G4 = 4 * H                  # 4096 gate rows
NM = G4 // 128              # 32 gate m-tiles
NKH = H // 128              # 8 hidden K-chunks
NKX = IN // 128             # 16 input K-chunks (biases folded via ACT bias)
NQ = 4                      # wih quad-chunks per sweep (4 kc each)
NSW = 4                     # sweeps; sweep s covers hc pair (2s, 2s+1)
INIT = 0.01

LAST_EXEC_NS = None
LAST_RESULTS = None

_cached_nc = None


def _build():
    nc = bacc.Bacc(None, target_bir_lowering=False)
    wih = nc.dram_tensor("wih", [NSW, NQ, 128, 4 * 1024], F16, kind="ExternalInput")
    whh = nc.dram_tensor("whh", [NKH, 128, G4], F16, kind="ExternalInput")
    xt = nc.dram_tensor("xt", [NKX, 128, BL], F16, kind="ExternalInput")
    rb = nc.dram_tensor("rb", [128, NM], F32, kind="ExternalInput")
    bb = nc.dram_tensor("bb", [128, NM], F32, kind="ExternalInput")
    hs = nc.dram_tensor("hs", [T, 128, NKH * BL], F16, kind="ExternalOutput")

    with tile.TileContext(nc) as tc:
        with (
            tc.tile_pool(name="const", bufs=1) as constp,
            tc.tile_pool(name="wihp", bufs=4) as wihp,
            tc.tile_pool(name="w0p", bufs=2) as w0p,
            tc.tile_pool(name="state", bufs=2) as statep,
            tc.tile_pool(name="gates", bufs=3) as gatesp,
            tc.tile_pool(name="psum", bufs=8, space="PSUM") as psump,
        ):
            xt_sbq = [
                constp.tile([128, 4 * BL], F16, tag=f"xtq{q}", name=f"xtq{q}")
                for q in range(4)
            ]
            r_sb = constp.tile([128, NM], F32, tag="rb")
            bb_sb = constp.tile([128, NM], F32, tag="bb")
            xg_sb = constp.tile([128, NM * BL], F32, tag="xg")
            whh_sb = [
                constp.tile([128, G4], F16, tag=f"whh{kc}", name=f"whh{kc}")
                for kc in range(NKH)
            ]

            # Sweep 0's wih as 16 single-kc tiles interleaved with the x
            # quarters: the DMA unit transfers serially (~0.7us per 256KB),
            # so fine-grained tiles keep delivery just ahead of the PE's
            # ~0.93us/chunk consumption from the very first matmul.
            w0s = [
                w0p.tile([128, 1024], F16, tag=f"w0s{i % 4}", name=f"w0s_{i}")
                for i in range(NKX)
            ]

            def _dma_w0(i):
                nc.sync.dma_start(
                    w0s[i][:, :], wih[0, i // 4][:, (i % 4) * 1024:
                                                 (i % 4 + 1) * 1024]
                )

            def _dma_xtq(q):
                nc.sync.dma_start(
                    xt_sbq[q][:, :].rearrange("p (kc b) -> p kc b", kc=4),
                    xt[4 * q:4 * q + 4].rearrange("kc p b -> p kc b"),
                )

            _dma_w0(0)
            _dma_xtq(0)
            for i in (1, 2, 3):
                _dma_w0(i)
            _dma_xtq(1)
            for i in (4, 5, 6, 7):
                _dma_w0(i)
            _dma_xtq(2)
            for i in (8, 9, 10, 11):
                _dma_w0(i)
            _dma_xtq(3)
            for i in (12, 13, 14, 15):
                _dma_w0(i)
            # first sweep-1 quad jumps ahead of r/bb (needed ~3us earlier)
            wq1 = wihp.tile([128, 4 * 1024], F16, tag="wih", name="wq1")
            nc.sync.dma_start(wq1[:, :], wih[1, 0])
            nc.sync.dma_start(r_sb[:, :], rb[:, :])
            nc.sync.dma_start(bb_sb[:, :], bb[:, :])

            # ---- Phase A: xg = W_ih @ x (single fp16 pass) fused with
            # step 0 (per-partition ACT bias carries b_ih+b_hh and the
            # constant-h0 recurrent term 0.01*rowsum(W_hh)).
            h0 = statep.tile([128, NKH * BL], F16, tag="h")
            c0 = statep.tile([128, NKH * BL], F32, tag="c")
            for s in range(NSW):
                # one [128,256] PSUM tile per (gate, hc-of-pair): a PSUM bank
                # admits only ONE pending accumulation group (2KB zero
                # region), so tiles must not share banks across groups.
                pst = [
                    psump.tile([128, BL], F32, tag="ps", name=f"psA{i}")
                    for i in range(8)
                ]
                for qc in range(NQ):
                    first = s == 0
                    if not first:
                        if s == 1 and qc == 0:
                            wq = wq1
                        else:
                            wq = wihp.tile(
                                [128, 4 * 1024], F16, tag="wih", name="wq"
                            )
                            nc.sync.dma_start(wq[:, :], wih[s, qc])
                    for kci in range(4):
                        kc = qc * 4 + kci
                        for ml in range(8):
                            lhs = (
                                w0s[kc][:, ml * 128:(ml + 1) * 128]
                                if first else
                                wq[:, kci * 1024 + ml * 128:
                                   kci * 1024 + (ml + 1) * 128]
                            )
                            nc.tensor.matmul(
                                pst[ml][:, :],
                                lhs,
                                xt_sbq[qc][:, kci * BL:(kci + 1) * BL],
                                start=(kc == 0),
                                stop=(kc == NKX - 1),
                            )
                g0 = {}
                for ml in range(8):
                    gi, j = ml // 2, ml % 2
                    q = (2 * s + j) * 4 + gi
                    # xg tile saved with biases folded in; on DVE (idle in
                    # phase A) so the PSUM slot frees without waiting on the
                    # ACT queue at sweep boundaries
                    nc.vector.tensor_scalar_add(
                        xg_sb[:, q * BL:(q + 1) * BL],
                        pst[ml][:, :],
                        bb_sb[:, q:q + 1],
                    )
                    g = gatesp.tile([128, BL], F32, tag=f"g{gi}", name=f"gA{gi}{j}")
                    fn = AF.Tanh if gi == 2 else AF.Sigmoid
                    nc.scalar.activation(
                        g[:, :], pst[ml][:, :], fn,
                        bias=r_sb[:, q:q + 1],
                    )
                    g0[(gi, j)] = g
                for j in range(2):
                    hc = 2 * s + j
                    sl = slice(hc * BL, (hc + 1) * BL)
                    t0 = gatesp.tile([128, BL], F32, tag="t0")
                    th = gatesp.tile([128, BL], F32, tag="th")
                    nc.vector.tensor_mul(t0[:, :], g0[(0, j)][:, :], g0[(2, j)][:, :])
                    # c0 = f*INIT + i*g  (c_prev is the 0.01 constant)
                    nc.vector.scalar_tensor_tensor(
                        c0[:, sl], g0[(1, j)][:, :], INIT, t0[:, :],
                        ALU.mult, ALU.add,
                    )
                    nc.scalar.activation(th[:, :], c0[:, sl], AF.Tanh)
                    nc.vector.tensor_mul(h0[:, sl], g0[(3, j)][:, :], th[:, :])
            # W_hh streams in right after the wih tiles, earliest chunk first
            for kc in range(NKH):
                nc.sync.dma_start(whh_sb[kc][:, :], whh[kc])
            nc.sync.dma_start(hs[0], h0[:, :])

            # ---- Phase B: recurrent steps 1..14 ----
            def _cell(gt, hc, c_prev, c_new, h_new):
                sl = slice(hc * BL, (hc + 1) * BL)
                t0 = gatesp.tile([128, BL], F32, tag="t0", name="t0")
                t1 = gatesp.tile([128, BL], F32, tag="t1", name="t1")
                th = gatesp.tile([128, BL], F32, tag="th", name="th")
                nc.vector.tensor_mul(t0[:, :], gt[0][:, :], gt[2][:, :])
                nc.vector.tensor_mul(t1[:, :], gt[1][:, :], c_prev[:, sl])
                nc.vector.tensor_add(c_new[:, sl], t0[:, :], t1[:, :])
                nc.scalar.activation(th[:, :], c_new[:, sl], AF.Tanh)
                nc.vector.tensor_mul(h_new[:, sl], gt[3][:, :], th[:, :])

            h_prev, c_prev = h0, c0

            for t in range(1, T):
                h_new = statep.tile([128, NKH * BL], F16, tag="h")
                c_new = statep.tile([128, NKH * BL], F32, tag="c")
                for hc in range(NKH):
                    pss = []
                    for gi in range(4):
                        m = gi * NKH + hc
                        ps = psump.tile([128, BL], F32, tag="ps", name="psB")
                        # For the first hc group of a step, defer the kc=7
                        # chunk: the previous step's h[7] lands ~1.3us after
                        # its last matmul, so give the PE runway.
                        kcs = (
                            list(range(NKH - 1)) if hc == 0
                            else list(range(NKH))
                        )
                        for kc in kcs:
                            nc.tensor.matmul(
                                ps[:, :],
                                whh_sb[kc][:, m * 128:(m + 1) * 128],
                                h_prev[:, kc * BL:(kc + 1) * BL],
                                start=(kc == 0),
                                stop=(kc == NKH - 1),
                            )
                        pss.append(ps)
                    if hc == 0:
                        for gi in range(4):
                            m = gi * NKH
                            nc.tensor.matmul(
                                pss[gi][:, :],
                                whh_sb[NKH - 1][:, m * 128:(m + 1) * 128],
                                h_prev[:, (NKH - 1) * BL:NKH * BL],
                                start=False,
                                stop=True,
                            )
                    gt = []
                    for gi in range(4):
                        q = hc * 4 + gi
                        ps = pss[gi]
                        nc.vector.tensor_add(
                            ps[:, :], ps[:, :], xg_sb[:, q * BL:(q + 1) * BL]
                        )
                        g = gatesp.tile([128, BL], F32, tag=f"g{gi}", name=f"gB{gi}")
                        fn = AF.Tanh if gi == 2 else AF.Sigmoid
                        nc.scalar.activation(g[:, :], ps[:, :], fn)
                        gt.append(g)
                    _cell(gt, hc, c_prev, c_new, h_new)
                    if t == T - 1 and hc == 3:
                        # last step: flush the first half of hs early so the
                        # end-of-kernel drain only waits on half a tile
                        nc.sync.dma_start(
                            hs[t][:, :4 * BL], h_new[:, :4 * BL]
                        )
                if t == T - 1:
                    nc.sync.dma_start(hs[t][:, 4 * BL:], h_new[:, 4 * BL:])
                else:
                    nc.sync.dma_start(hs[t], h_new[:, :])
                h_prev, c_prev = h_new, c_new

    nc.compile()
    return nc


def timeline_ns():
    from concourse.timeline_sim import TimelineSim
    nc = _get_nc()
    ts = TimelineSim(nc)
    ts.simulate()
    return ts.time


def _get_nc():
    global _cached_nc
    if _cached_nc is None:
        _cached_nc = _build()
    return _cached_nc


def _pack_weights(W_ih, W_hh, b_ih, b_hh):
    f16 = np.float16
    wt = W_ih.T.astype(np.float32)                          # [IN, 4H]
    # [kc, p, gi, s, j, col] -> [s, kc, p, gi, j, col]
    wperm = wt.reshape(NKX, 128, 4, NSW, 2, 128).transpose(3, 0, 1, 2, 4, 5)
    # regroup kc into quads: [s, qc, kci, p, gi, j, col] -> [s, qc, p, kci, ...]
    wperm = wperm.reshape(NSW, NQ, 4, 128, 1024).transpose(0, 1, 3, 2, 4)
    wih_host = np.ascontiguousarray(
        wperm.reshape(NSW, NQ, 128, 4 * 1024)
    ).astype(f16)
    whh_host = np.ascontiguousarray(W_hh.T.reshape(NKH, 128, G4)).astype(f16)
    bias = (b_ih + b_hh).astype(np.float32)                 # [4096]
    r = (INIT * W_hh.sum(axis=1)).astype(np.float32) + bias

    def _qlay(v):  # [4096] in (gi, hc, p) order -> [128, q=hc*4+gi]
        return np.ascontiguousarray(
            v.reshape(4, NKH, 128).transpose(1, 0, 2).reshape(NM, 128).T
        )

    return wih_host, whh_host, _qlay(r), _qlay(bias)


def kernel(x, W_ih, W_hh, b_ih, b_hh):
    global LAST_EXEC_NS, LAST_RESULTS
    nc = _get_nc()
    x = np.asarray(x, np.float32)
    W_ih = np.asarray(W_ih, np.float32)
    W_hh = np.asarray(W_hh, np.float32)
    b_ih = np.asarray(b_ih, np.float32)
    b_hh = np.asarray(b_hh, np.float32)

    wih_host, whh_host, r_host, b_host = _pack_weights(W_ih, W_hh, b_ih, b_hh)

    in_maps = []
    for c in range(NCORES):
        xa = np.ascontiguousarray(x[c * BL:(c + 1) * BL].T)
        xt_host = xa.astype(np.float16).reshape(NKX, 128, BL)
        in_maps.append({
            "wih": wih_host, "whh": whh_host, "xt": xt_host,
            "rb": r_host, "bb": b_host,
        })

    trace = os.environ.get("LSTM_TRACE") == "1"
    res = run_bass_kernel_spmd(
        nc, in_maps, core_ids=list(range(NCORES)), trace=trace
    )
    LAST_EXEC_NS = res.exec_time_ns
    LAST_RESULTS = res

    out = np.empty((T, B, H), np.float32)
    for c in range(NCORES):
        a = res.results[c]["hs"].astype(np.float32).reshape(T, 128, NKH, BL)
        out[:, c * BL:(c + 1) * BL, :] = (
            a.transpose(0, 3, 2, 1).reshape(T, BL, H)
        )
    return out
